# revision 18
# baseline (speedup 1.0000x reference)
"""Multi-head attention (B=2, T=2048, D=1024, H=16, d_k=64) on 8 trn2 cores.

Sharding: tensor-parallel over heads (4 TP groups of 4 heads) x data-parallel
over batch (2). Core c handles batch (c // 4) and heads [4*(c%4), 4*(c%4)+4).

Per-core device program (all matmuls bf16 inputs, fp32 PSUM accumulate):
  QT/KT = W @ X^T            [256, 2048] (d-major layout, + bias via DVE)
  V     = X @ Wv^T           [2048, 256] (s-major, ones column appended)
  ST    = K^T-slices^T Q^T   scores transposed [s, t]; softmax runs over the
                             partition dim implicitly:
  P     = exp(ST / 32)       (ACT, no max-subtraction: |ST/32| < ~1)
  O'/r  = [V|1]^T P          one matmul gives both the unnormalized output and
                             the softmax denominator r (row 64)
  O     = O' * (1/r)         (DVE, gpsimd partition_broadcast for 1/r)
  Y     = O^T @ Wo-slice     [2048, 1024] partial, summed across TP on host.

Host folds the V bias and output bias: attn rows sum to 1, so
out += bo + bv @ Wo^T once per batch after the TP reduction.
"""

import numpy as np
import ml_dtypes

import concourse.bass as bass
from concourse import bacc
import concourse.mybir as mybir
import concourse.tile as tile
from concourse.bass_utils import run_bass_kernel_spmd

# ---------------------------------------------------------------------------

P = 128
T = 2048          # sequence length
DM = 1024         # d_model
DC = 256          # per-core head dims (4 heads x 64)
NH = 4            # heads per core
DK = 64
ET = DM // P      # 8 contraction tiles
TT = T // P       # 16 s/t tiles
N_CORES = 8
BF16 = mybir.dt.bfloat16
F32 = mybir.dt.float32
F32R = mybir.dt.float32r
SCALE = 1.0 / 32.0  # 1/sqrt(d_model)

_ts = bass.ts


def build_nc():
    nc = bacc.Bacc("TRN2", target_bir_lowering=False, debug=False)
    xT = nc.dram_tensor("xT", (DM, T), BF16, kind="ExternalInput")
    wq = nc.dram_tensor("wq", (DM, DC), BF16, kind="ExternalInput")
    wk = nc.dram_tensor("wk", (DM, DC), BF16, kind="ExternalInput")
    wv = nc.dram_tensor("wv", (DM, DC), BF16, kind="ExternalInput")
    wo = nc.dram_tensor("wo", (DC, DM), BF16, kind="ExternalInput")
    bqk = nc.dram_tensor("bqk", (P, 4), F32, kind="ExternalInput")
    onec = nc.dram_tensor("onec", (P, TT * NH), BF16, kind="ExternalInput")
    onesr = nc.dram_tensor("onesr", (1, DK), F32R, kind="ExternalInput")
    y = nc.dram_tensor("y", (T, DM), F32, kind="ExternalOutput")

    with tile.TileContext(nc) as tc:
        from contextlib import ExitStack
        with ExitStack() as ctx:
            cst = ctx.enter_context(tc.tile_pool(name="cst", bufs=1))
            # persistent SBUF tensors
            wq_sb = cst.tile([P, ET, DC], BF16, tag="wq")
            wk_sb = cst.tile([P, ET, DC], BF16, tag="wk")
            wv_sb = cst.tile([P, ET, DC], BF16, tag="wv")
            wo_sb = cst.tile([P, 2, DM], BF16, tag="wo")
            bqk_sb = cst.tile([P, 4], F32, tag="bqk")
            ones_r = cst.tile([1, DK], F32R, tag="ones_r")
            qt_sb = cst.tile([P, 2, T], BF16, tag="qt")
            kt_sb = cst.tile([P, 2, T], BF16, tag="kt")
            v_sb = cst.tile([P, TT, NH, DK + 1], BF16, tag="v")
            ot_sb = cst.tile([P, 2, T], BF16, tag="ot")

            nc.sync.dma_start(wq_sb[:], wq[:].rearrange("(e p) d -> p e d", p=P))
            nc.sync.dma_start(wk_sb[:], wk[:].rearrange("(e p) d -> p e d", p=P))
            nc.sync.dma_start(wv_sb[:], wv[:].rearrange("(e p) d -> p e d", p=P))
            nc.sync.dma_start(wo_sb[:], wo[:].rearrange("(g p) f -> p g f", p=P))
            nc.sync.dma_start(bqk_sb[:], bqk[:])
            nc.sync.dma_start(
                v_sb[:, :, :, DK:DK + 1],
                onec[:].rearrange("p (s h) -> p s h", s=TT, h=NH).unsqueeze(-1))
            nc.sync.dma_start(ones_r[:], onesr[:])

            ps_big = ctx.enter_context(
                tc.tile_pool(name="ps_big", bufs=3, space="PSUM"))
            ps_sml = ctx.enter_context(
                tc.tile_pool(name="ps_sml", bufs=2, space="PSUM"))

            # ---- phase B: projections (uses xT; scoped so its SBUF frees) --
            with tc.tile_pool(name="xtp", bufs=1) as xtp:
                xt_sb = xtp.tile([P, ET, T], BF16, tag="xt")
                nc.sync.dma_start(xt_sb[:], xT[:].rearrange("(e p) t -> p e t", p=P))

                # QT / KT: d-major [256, 2048], bias added on the PSUM drain
                for w_sb, dst, bcol in ((wq_sb, qt_sb, 0), (wk_sb, kt_sb, 2)):
                    for dt in range(2):
                        for tcp in range(2):  # 1024-wide chunks
                            ps = ps_big.tile([P, 1024], F32, tag="stps")
                            for u in range(2):
                                t0 = tcp * 1024 + u * 512
                                for e in range(ET):
                                    nc.tensor.matmul(
                                        ps[:, u * 512:(u + 1) * 512],
                                        w_sb[:, e, dt * P:(dt + 1) * P],
                                        xt_sb[:, e, t0:t0 + 512],
                                        start=(e == 0), stop=(e == ET - 1))
                            nc.vector.tensor_scalar_add(
                                dst[:, dt, tcp * 1024:(tcp + 1) * 1024], ps[:],
                                bqk_sb[:, bcol + dt:bcol + dt + 1])

                # V: s-major [2048, 256] (+ ones col), X^T tiles stationary
                for sq in range(TT // 2):
                    ps = ps_sml.tile([P, 512], F32, tag="ps512")
                    for st2 in range(2):
                        st = sq * 2 + st2
                        for e in range(ET):
                            nc.tensor.matmul(
                                ps[:, st2 * 256:(st2 + 1) * 256],
                                xt_sb[:, e, st * P:(st + 1) * P],
                                wv_sb[:, e, :],
                                start=(e == 0), stop=(e == ET - 1))
                    nc.vector.tensor_copy(
                        v_sb[:, sq * 2:(sq + 1) * 2, :, 0:DK],
                        ps[:].rearrange("p (s h d) -> p s h d", s=2, h=NH))

            # ---- phase C: attention, one head-pair at a time ---------------
            ptp = ctx.enter_context(tc.tile_pool(name="ptp", bufs=32))
            rsp = ctx.enter_context(tc.tile_pool(name="rsp", bufs=2))
            bcs = ctx.enter_context(tc.tile_pool(name="bcs", bufs=3))
            o64 = ctx.enter_context(tc.tile_pool(name="o64", bufs=3))
            ysp = ctx.enter_context(tc.tile_pool(name="ysp", bufs=3))

            for hp in range(2):
                # scores^T + exp, both heads of the pair interleaved so the
                # K=64 matmuls pack into disjoint PE row groups
                pt = [[None] * TT, [None] * TT]
                for st in range(TT):
                    for tch in range(2):
                        pspair = []
                        for h2 in range(2):
                            if tch == 0:
                                pt[h2][st] = ptp.tile([P, T], BF16, tag="pt", name="pt")
                            pspair.append(ps_big.tile([P, 1024], F32, tag="stps", name="stps"))
                        for u in range(2):
                            t0 = tch * 1024 + u * 512
                            for h2 in range(2):
                                r0 = h2 * DK
                                nc.tensor.matmul(
                                    pspair[h2][:, u * 512:(u + 1) * 512],
                                    kt_sb[r0:r0 + DK, hp, st * P:(st + 1) * P],
                                    qt_sb[r0:r0 + DK, hp, t0:t0 + 512],
                                    start=True, stop=True)
                        for h2 in range(2):
                            nc.scalar.activation(
                                pt[h2][st][:, tch * 1024:(tch + 1) * 1024],
                                pspair[h2][:],
                                mybir.ActivationFunctionType.Exp, scale=SCALE)

                # attention @ [V|1], then normalize by r and stage into ot_sb
                for h2 in range(2):
                    h = hp * 2 + h2
                    for tc4 in range(4):  # one 512-wide t chunk per pass
                        av = ps_sml.tile([P, 512], F32, tag="ps512", name="avps")
                        for st in range(TT):
                            nc.tensor.matmul(
                                av[0:DK + 1, :],
                                v_sb[:, st, h, :],
                                pt[h2][st][:, tc4 * 512:(tc4 + 1) * 512],
                                start=(st == 0), stop=(st == TT - 1))
                        # softmax denominator r sits in row 64; move it to
                        # partition 0 (DMA hop), invert, broadcast via PE.
                        rv = rsp.tile([DK + 1, 512], F32R, tag="rv", name="rv")
                        with nc.allow_low_precision(reason="softmax recip"):
                            nc.vector.reciprocal(
                                rv[DK:DK + 1, :], av[DK:DK + 1, :])
                        rv0 = rsp.tile([1, 512], F32R, tag="rv0", name="rv0")
                        nc.sync.dma_start(rv0[0:1, :], rv[DK:DK + 1, :])
                        bc = ps_sml.tile([DK, 512], F32, tag="ps512", name="bcps")
                        nc.tensor.matmul(bc[:], ones_r[:], rv0[:],
                                         start=True, stop=True)
                        bcs_t = bcs.tile([DK, 512], F32, tag="bcs", name="bcs_t")
                        nc.vector.tensor_copy(bcs_t[:], bc[:])
                        ob = o64.tile([DK, 512], BF16, tag="ob", name="ob")
                        nc.vector.tensor_tensor(
                            ob[:], av[0:DK, :], bcs_t[:],
                            mybir.AluOpType.mult)
                        nc.sync.dma_start(
                            ot_sb[h2 * DK:(h2 + 1) * DK, hp,
                                  tc4 * 512:(tc4 + 1) * 512],
                            ob[:])

            # ---- phase D: output projection -------------------------------
            for tt in range(TT):
                for n2 in range(2):
                    ps = ps_sml.tile([P, 512], F32, tag="ps512")
                    for dt in range(2):
                        nc.tensor.matmul(
                            ps[:],
                            ot_sb[:, dt, tt * P:(tt + 1) * P],
                            wo_sb[:, dt, n2 * 512:(n2 + 1) * 512],
                            start=(dt == 0), stop=(dt == 1))
                    ysb = ysp.tile([P, 512], F32, tag="ysb")
                    nc.vector.tensor_copy(ysb[:], ps[:])
                    nc.sync.dma_start(
                        y[tt * P:(tt + 1) * P, n2 * 512:(n2 + 1) * 512], ysb[:])
    nc.compile()
    return nc


_NC_CACHE = None


def _get_nc():
    global _NC_CACHE
    if _NC_CACHE is None:
        _NC_CACHE = build_nc()
    return _NC_CACHE


def _prep_inputs(x, Wq, bq, Wk, bk, Wv, bv, Wo, bo):
    bf = ml_dtypes.bfloat16
    in_maps = []
    for c in range(N_CORES):
        b, hg = c // 4, c % 4
        sl = slice(hg * DC, (hg + 1) * DC)
        bqk = np.empty((P, 4), np.float32)
        bqk[:, 0] = bq[sl][0:P]
        bqk[:, 1] = bq[sl][P:DC]
        bqk[:, 2] = bk[sl][0:P]
        bqk[:, 3] = bk[sl][P:DC]
        in_maps.append({
            "xT": np.ascontiguousarray(x[b].T).astype(bf),
            "wq": np.ascontiguousarray(Wq[sl, :].T).astype(bf),
            "wk": np.ascontiguousarray(Wk[sl, :].T).astype(bf),
            "wv": np.ascontiguousarray(Wv[sl, :].T).astype(bf),
            "wo": np.ascontiguousarray(Wo[:, sl].T).astype(bf),
            "bqk": bqk,
            "onec": np.ones((P, TT * NH), bf),
            "onesr": np.ones((1, DK), np.float32),
        })
    return in_maps


def _gather(results, Wo, bv, bo):
    bias = bo.astype(np.float64) + bv.astype(np.float64) @ Wo.T.astype(np.float64)
    out = np.empty((2, T, DM), np.float32)
    for b in range(2):
        acc = np.zeros((T, DM), np.float64)
        for hg in range(4):
            acc += results[b * 4 + hg]["y"]
        out[b] = (acc + bias).astype(np.float32)
    return out


def kernel(x, Wq, bq, Wk, bk, Wv, bv, Wo, bo, _trace=False, _res_box=None):
    x = np.asarray(x, np.float32)
    Wq, bq = np.asarray(Wq, np.float32), np.asarray(bq, np.float32)
    Wk, bk = np.asarray(Wk, np.float32), np.asarray(bk, np.float32)
    Wv, bv = np.asarray(Wv, np.float32), np.asarray(bv, np.float32)
    Wo, bo = np.asarray(Wo, np.float32), np.asarray(bo, np.float32)

    nc = _get_nc()
    in_maps = _prep_inputs(x, Wq, bq, Wk, bk, Wv, bv, Wo, bo)
    res = run_bass_kernel_spmd(nc, in_maps, core_ids=list(range(N_CORES)),
                               trace=_trace)
    if _res_box is not None:
        _res_box.append(res)
    return _gather(res.results, Wo, bv, bo)


# revision 19
# speedup vs baseline: 1.0917x; 1.0917x over previous
"""Multi-head attention (B=2, T=2048, D=1024, H=16, d_k=64) on 8 trn2 cores.

Sharding: tensor-parallel over heads (4 TP groups of 4 heads) x data-parallel
over batch (2). Core c handles batch (c // 4) and heads [4*(c%4), 4*(c%4)+4).

Per-core device program (all matmuls bf16 inputs, fp32 PSUM accumulate):
  QT/KT = W @ X^T            [256, 2048] (d-major layout, + bias via DVE)
  V     = X @ Wv^T           [2048, 256] (s-major, ones column appended)
  ST    = K^T-slices^T Q^T   scores transposed [s, t]; softmax runs over the
                             partition dim implicitly:
  P     = exp(ST / 32)       (ACT, no max-subtraction: |ST/32| < ~1)
  O'/r  = [V|1]^T P          one matmul gives both the unnormalized output and
                             the softmax denominator r (row 64)
  O     = O' * (1/r)         (DVE, gpsimd partition_broadcast for 1/r)
  Y     = O^T @ Wo-slice     [2048, 1024] partial, summed across TP on host.

Host folds the V bias and output bias: attn rows sum to 1, so
out += bo + bv @ Wo^T once per batch after the TP reduction.
"""

import numpy as np
import ml_dtypes

import concourse.bass as bass
from concourse import bacc
import concourse.mybir as mybir
import concourse.tile as tile
from concourse.bass_utils import run_bass_kernel_spmd

# ---------------------------------------------------------------------------

P = 128
T = 2048          # sequence length
DM = 1024         # d_model
DC = 256          # per-core head dims (4 heads x 64)
NH = 4            # heads per core
DK = 64
ET = DM // P      # 8 contraction tiles
TT = T // P       # 16 s/t tiles
N_CORES = 8
BF16 = mybir.dt.bfloat16
F32 = mybir.dt.float32
F32R = mybir.dt.float32r
SCALE = 1.0 / 32.0  # 1/sqrt(d_model)

_ts = bass.ts


def build_nc():
    nc = bacc.Bacc("TRN2", target_bir_lowering=False, debug=False)
    xT = nc.dram_tensor("xT", (DM, T), BF16, kind="ExternalInput")
    wq = nc.dram_tensor("wq", (DM, DC), BF16, kind="ExternalInput")
    wk = nc.dram_tensor("wk", (DM, DC), BF16, kind="ExternalInput")
    wv = nc.dram_tensor("wv", (DM, DC), BF16, kind="ExternalInput")
    wo = nc.dram_tensor("wo", (DC, DM), BF16, kind="ExternalInput")
    bqk = nc.dram_tensor("bqk", (P, 4), F32, kind="ExternalInput")
    onec = nc.dram_tensor("onec", (P, TT * NH), BF16, kind="ExternalInput")
    onesr = nc.dram_tensor("onesr", (1, DK), F32R, kind="ExternalInput")
    y = nc.dram_tensor("y", (T, DM), F32, kind="ExternalOutput")

    with tile.TileContext(nc) as tc:
        from contextlib import ExitStack
        with ExitStack() as ctx:
            cst = ctx.enter_context(tc.tile_pool(name="cst", bufs=1))
            # persistent SBUF tensors
            wq_sb = cst.tile([P, ET, DC], BF16, tag="wq")
            wk_sb = cst.tile([P, ET, DC], BF16, tag="wk")
            wv_sb = cst.tile([P, ET, DC], BF16, tag="wv")
            wo_sb = cst.tile([P, 2, DM], BF16, tag="wo")
            bqk_sb = cst.tile([P, 4], F32, tag="bqk")
            ones_r = cst.tile([1, DK], F32R, tag="ones_r")
            qt_sb = cst.tile([P, 2, T], BF16, tag="qt")
            kt_sb = cst.tile([P, 2, T], BF16, tag="kt")
            v_sb = cst.tile([P, TT, NH, DK + 1], BF16, tag="v")
            ot_sb = cst.tile([P, 2, T], BF16, tag="ot")

            nc.sync.dma_start(wq_sb[:], wq[:].rearrange("(e p) d -> p e d", p=P))
            nc.sync.dma_start(wk_sb[:], wk[:].rearrange("(e p) d -> p e d", p=P))
            nc.sync.dma_start(wv_sb[:], wv[:].rearrange("(e p) d -> p e d", p=P))
            nc.sync.dma_start(wo_sb[:], wo[:].rearrange("(g p) f -> p g f", p=P))
            nc.sync.dma_start(bqk_sb[:], bqk[:])
            nc.sync.dma_start(
                v_sb[:, :, :, DK:DK + 1],
                onec[:].rearrange("p (s h) -> p s h", s=TT, h=NH).unsqueeze(-1))
            nc.sync.dma_start(ones_r[:], onesr[:])

            ps_big = ctx.enter_context(
                tc.tile_pool(name="ps_big", bufs=3, space="PSUM"))
            ps_sml = ctx.enter_context(
                tc.tile_pool(name="ps_sml", bufs=2, space="PSUM"))

            # ---- phase B: projections (uses xT; scoped so its SBUF frees) --
            with tc.tile_pool(name="xtp", bufs=1) as xtp:
                xt_sb = xtp.tile([P, ET, T], BF16, tag="xt")
                nc.sync.dma_start(xt_sb[:], xT[:].rearrange("(e p) t -> p e t", p=P))

                # QT / KT: d-major [256, 2048], bias added on the PSUM drain
                for w_sb, dst, bcol in ((wq_sb, qt_sb, 0), (wk_sb, kt_sb, 2)):
                    for dt in range(2):
                        for tcp in range(2):  # 1024-wide chunks
                            ps = ps_big.tile([P, 1024], F32, tag="stps")
                            for u in range(2):
                                t0 = tcp * 1024 + u * 512
                                for e in range(ET):
                                    nc.tensor.matmul(
                                        ps[:, u * 512:(u + 1) * 512],
                                        w_sb[:, e, dt * P:(dt + 1) * P],
                                        xt_sb[:, e, t0:t0 + 512],
                                        start=(e == 0), stop=(e == ET - 1))
                            nc.vector.tensor_scalar_add(
                                dst[:, dt, tcp * 1024:(tcp + 1) * 1024], ps[:],
                                bqk_sb[:, bcol + dt:bcol + dt + 1])

                # V: s-major [2048, 256] (+ ones col), X^T tiles stationary
                for sq in range(TT // 2):
                    ps = ps_sml.tile([P, 512], F32, tag="ps512")
                    for st2 in range(2):
                        st = sq * 2 + st2
                        for e in range(ET):
                            nc.tensor.matmul(
                                ps[:, st2 * 256:(st2 + 1) * 256],
                                xt_sb[:, e, st * P:(st + 1) * P],
                                wv_sb[:, e, :],
                                start=(e == 0), stop=(e == ET - 1))
                    nc.vector.tensor_copy(
                        v_sb[:, sq * 2:(sq + 1) * 2, :, 0:DK],
                        ps[:].rearrange("p (s h d) -> p s h d", s=2, h=NH))

            # ---- phase C: attention, one head-pair at a time ---------------
            ptp = ctx.enter_context(tc.tile_pool(name="ptp", bufs=32))
            rsp = ctx.enter_context(tc.tile_pool(name="rsp", bufs=2))
            bcs = ctx.enter_context(tc.tile_pool(name="bcs", bufs=3))
            o64 = ctx.enter_context(tc.tile_pool(name="o64", bufs=3))
            ysp = ctx.enter_context(tc.tile_pool(name="ysp", bufs=3))

            for hp in range(2):
                pt = [[None] * TT, [None] * TT]
                for h2 in range(2):
                    h = hp * 2 + h2
                    r0 = h2 * DK
                    # scores^T + exp for this head; the previous head's AV
                    # matmuls below keep the PE busy while ACT works
                    for st in range(TT):
                        for tch in range(2):
                            if tch == 0:
                                pt[h2][st] = ptp.tile([P, T], BF16, tag="pt", name="pt")
                            ps = ps_big.tile([P, 1024], F32, tag="stps", name="stps")
                            for u in range(2):
                                t0 = tch * 1024 + u * 512
                                nc.tensor.matmul(
                                    ps[:, u * 512:(u + 1) * 512],
                                    kt_sb[r0:r0 + DK, hp, st * P:(st + 1) * P],
                                    qt_sb[r0:r0 + DK, hp, t0:t0 + 512],
                                    start=True, stop=True)
                            nc.scalar.activation(
                                pt[h2][st][:, tch * 1024:(tch + 1) * 1024],
                                ps[:],
                                mybir.ActivationFunctionType.Exp, scale=SCALE)
                    # attention @ [V|1] for this head
                    for tc4 in range(4):  # one 512-wide t chunk per pass
                        av = ps_sml.tile([P, 512], F32, tag="ps512", name="avps")
                        for st in range(TT):
                            nc.tensor.matmul(
                                av[0:DK + 1, :],
                                v_sb[:, st, h, :],
                                pt[h2][st][:, tc4 * 512:(tc4 + 1) * 512],
                                start=(st == 0), stop=(st == TT - 1))
                        # softmax denominator r sits in row 64; move it to
                        # partition 0 (DMA hop), invert, broadcast via PE.
                        rv = rsp.tile([DK + 1, 512], F32R, tag="rv", name="rv")
                        with nc.allow_low_precision(reason="softmax recip"):
                            nc.vector.reciprocal(
                                rv[DK:DK + 1, :], av[DK:DK + 1, :])
                        rv0 = rsp.tile([1, 512], F32R, tag="rv0", name="rv0")
                        nc.sync.dma_start(rv0[0:1, :], rv[DK:DK + 1, :])
                        bc = ps_sml.tile([DK, 512], F32, tag="ps512", name="bcps")
                        nc.tensor.matmul(bc[:], ones_r[:], rv0[:],
                                         start=True, stop=True)
                        bcs_t = bcs.tile([DK, 512], F32, tag="bcs", name="bcs_t")
                        nc.vector.tensor_copy(bcs_t[:], bc[:])
                        ob = o64.tile([DK, 512], BF16, tag="ob", name="ob")
                        nc.vector.tensor_tensor(
                            ob[:], av[0:DK, :], bcs_t[:],
                            mybir.AluOpType.mult)
                        nc.sync.dma_start(
                            ot_sb[h2 * DK:(h2 + 1) * DK, hp,
                                  tc4 * 512:(tc4 + 1) * 512],
                            ob[:])

            # ---- phase D: output projection -------------------------------
            for tt in range(TT):
                for n2 in range(2):
                    ps = ps_sml.tile([P, 512], F32, tag="ps512")
                    for dt in range(2):
                        nc.tensor.matmul(
                            ps[:],
                            ot_sb[:, dt, tt * P:(tt + 1) * P],
                            wo_sb[:, dt, n2 * 512:(n2 + 1) * 512],
                            start=(dt == 0), stop=(dt == 1))
                    ysb = ysp.tile([P, 512], F32, tag="ysb")
                    nc.vector.tensor_copy(ysb[:], ps[:])
                    nc.sync.dma_start(
                        y[tt * P:(tt + 1) * P, n2 * 512:(n2 + 1) * 512], ysb[:])
    nc.compile()
    return nc


_NC_CACHE = None


def _get_nc():
    global _NC_CACHE
    if _NC_CACHE is None:
        _NC_CACHE = build_nc()
    return _NC_CACHE


def _prep_inputs(x, Wq, bq, Wk, bk, Wv, bv, Wo, bo):
    bf = ml_dtypes.bfloat16
    in_maps = []
    for c in range(N_CORES):
        b, hg = c // 4, c % 4
        sl = slice(hg * DC, (hg + 1) * DC)
        bqk = np.empty((P, 4), np.float32)
        bqk[:, 0] = bq[sl][0:P]
        bqk[:, 1] = bq[sl][P:DC]
        bqk[:, 2] = bk[sl][0:P]
        bqk[:, 3] = bk[sl][P:DC]
        in_maps.append({
            "xT": np.ascontiguousarray(x[b].T).astype(bf),
            "wq": np.ascontiguousarray(Wq[sl, :].T).astype(bf),
            "wk": np.ascontiguousarray(Wk[sl, :].T).astype(bf),
            "wv": np.ascontiguousarray(Wv[sl, :].T).astype(bf),
            "wo": np.ascontiguousarray(Wo[:, sl].T).astype(bf),
            "bqk": bqk,
            "onec": np.ones((P, TT * NH), bf),
            "onesr": np.ones((1, DK), np.float32),
        })
    return in_maps


def _gather(results, Wo, bv, bo):
    bias = bo.astype(np.float64) + bv.astype(np.float64) @ Wo.T.astype(np.float64)
    out = np.empty((2, T, DM), np.float32)
    for b in range(2):
        acc = np.zeros((T, DM), np.float64)
        for hg in range(4):
            acc += results[b * 4 + hg]["y"]
        out[b] = (acc + bias).astype(np.float32)
    return out


def kernel(x, Wq, bq, Wk, bk, Wv, bv, Wo, bo, _trace=False, _res_box=None):
    x = np.asarray(x, np.float32)
    Wq, bq = np.asarray(Wq, np.float32), np.asarray(bq, np.float32)
    Wk, bk = np.asarray(Wk, np.float32), np.asarray(bk, np.float32)
    Wv, bv = np.asarray(Wv, np.float32), np.asarray(bv, np.float32)
    Wo, bo = np.asarray(Wo, np.float32), np.asarray(bo, np.float32)

    nc = _get_nc()
    in_maps = _prep_inputs(x, Wq, bq, Wk, bk, Wv, bv, Wo, bo)
    res = run_bass_kernel_spmd(nc, in_maps, core_ids=list(range(N_CORES)),
                               trace=_trace)
    if _res_box is not None:
        _res_box.append(res)
    return _gather(res.results, Wo, bv, bo)
